# revision 33
# baseline (speedup 1.0000x reference)
"""Trainium2 Bass kernel for AnchorProcessor (nms_detection).

Input  x: [8, 255, 128, 128] f32.  Output: [8, 18, 128, 128] f32.
Strategy: shard along H across 8 cores (16 rows each); per-core problem is
fully local (the buggy cross-batch max/argmax reduces over (N, cls), both
on-core), so no collectives.

v2 architecture (v1 in kernel_v1_backup.py ran 168us; DMA floor is ~80us):
  - Full-channel PE transposes: per (n, h) two transposes move channels
    [4:132) and [132:255) of x[n, :, h, :] into one PSUM tile [128pix, 252],
    so obj + all 3 anchors' logits arrive pixel-major together (256 big
    matmuls total vs 406 small ones in v1, and no separate obj transposes).
  - Input DMA as 16 big n-major tiles [128ch, 16h, 128w] with 8KB
    descriptors; compute pipelines per (n, h-quad) chunk behind the DMA.
  - negscore = (lg * -1) * obj is ONE scalar_tensor_tensor per chunk (obj
    columns broadcast along c) instead of v1's ACT/DVE per-(n,a) splits.
  - Work spread across engines (DVE was 116us in v1, gpsimd idle):
      mul: DVE n0-4, Pool n5, ACT n6-7 (per-partition obj scale)
      pack (bit-pack value|index): Pool (gpsimd)
      c-reduce (min over 80 classes, axis=X): DVE only (Pool can't do X)
      cross-n min + unpack: DVE (tiny)
      box path: ACT sigmoid/scale + Pool grid adds in a [96, 512] layout.
  - Same bit-pack argmax trick as v1: packed = (negscore&~0x3FF) | idx via
    (or,xor) stt; one min-reduce gives smax (2^-13 rel quant) + argmax.
"""

import os
import sys

for _p in ("/opt/trn_rl_repo", "/root/.axon_site/_ro/trn_rl_repo"):
    if _p not in sys.path:
        sys.path.append(_p)

import numpy as np

from concourse import bacc, masks, mybir
from concourse.tile import TileContext

N = 8          # batch
A = 3          # anchors
CLS = 80       # classes per anchor
W = 128        # width
HL = 16        # local H rows per core (128 / 8 cores)
NCORES = 8

ANCHOR_W = (116.0, 156.0, 373.0)
ANCHOR_H = (90.0, 198.0, 326.0)

F32 = mybir.dt.float32
U32 = mybir.dt.uint32

# engine assignment knobs (tunable via env for experiments)
def _env_tuple(name, default):
    v = os.environ.get(name)
    if v is None:
        return default
    return tuple(int(s) for s in v.split(",") if s != "")


MUL_DVE_NS = _env_tuple("MUL_DVE", ())   # mul on DVE (from SBUF copy)
MUL_ACT_NS = _env_tuple("MUL_ACT", ())   # mul fused on ACT (from PSUM)


def build_nc(hl=HL, reps=1, mul_dve=MUL_DVE_NS, mul_act=MUL_ACT_NS):
    import contextlib

    hqs = 4 if hl % 4 == 0 else hl     # h rows per chunk
    nch = hl // hqs                    # chunks per n
    bf = hl * W                        # box free size
    bp = A * N                         # box partitions

    nc = bacc.Bacc("TRN2", target_bir_lowering=False, debug=False)

    x = nc.declare_dram_parameter("x", [N, 255, hl, W], F32, isOutput=False)
    grid = nc.declare_dram_parameter("grid", [2, bp, bf], F32, isOutput=False)
    anch = nc.declare_dram_parameter("anch", [2, bp, 1], F32, isOutput=False)
    iota = nc.declare_dram_parameter("iota", [N * CLS], U32, isOutput=False)
    bits = nc.declare_dram_parameter("bits", [4], U32, isOutput=False)
    out = nc.declare_dram_parameter("out", [N, A * 6, hl, W], F32, isOutput=True)
    oscr = nc.dram_tensor("oscratch", [A * 2, hl, W], F32)

    with TileContext(nc) as tc:
        with (
            tc.tile_pool(name="const", bufs=1) as constp,
            tc.tile_pool(name="xt", bufs=1) as xp,
            tc.tile_pool(name="box", bufs=2) as boxp,
            tc.tile_pool(name="neg", bufs=2) as negp,
            tc.tile_pool(name="pak", bufs=2) as pakp,
            tc.tile_pool(name="red", bufs=1) as redp,
            tc.tile_pool(name="outsb", bufs=6) as outsbp,
            tc.tile_pool(name="ps", bufs=3, space="PSUM") as psp,
            tc.tile_pool(name="ps2", bufs=2, space="PSUM") as ps2p,
        ):
            ident = constp.tile([128, 128], F32)
            masks.make_identity(nc, ident[:, :])
            neg1 = constp.tile([128, 1], F32)
            nc.gpsimd.memset(neg1[:, :], -1.0)

            bitst = constp.tile([128, 4], U32)
            nc.sync.dma_start(
                out=bitst[:, :],
                in_=bits[:].unsqueeze(0).broadcast_to([128, 4]),
            )
            iotat = constp.tile([128, N * CLS], U32)
            nc.scalar.dma_start(
                out=iotat[:, :],
                in_=iota[:].unsqueeze(0).broadcast_to([128, N * CLS]),
            )
            gridt = [constp.tile([bp, bf], F32, name=f"grid{g}") for g in range(2)]
            ancht = [constp.tile([bp, 1], F32, name=f"anch{g}") for g in range(2)]
            for g in range(2):
                nc.scalar.dma_start(out=gridt[g][:, :], in_=grid[g, :, :])
                nc.scalar.dma_start(out=ancht[g][:, :], in_=anch[g, :, :])
            # (scalar-queue DMAs above are small; all bulk loads go on the
            # sync queue, whose descriptors spread across all 16 DMA engines)

            loop_cm = (
                tc.For_i(0, reps, 1, hint_engines=(mybir.EngineType.PE,))
                if reps > 1 else contextlib.nullcontext()
            )
            with loop_cm:
                body(nc, tc, x, out, oscr, hl, hqs, nch, bf, bp,
                     ident, bitst, iotat, gridt, ancht, neg1,
                     mul_dve, mul_act,
                     xp, boxp, negp, pakp, redp, outsbp, psp, ps2p)

    nc.compile()
    return nc


def body(nc, tc, x, out, oscr, hl, hqs, nch, bf, bp,
         ident, bitst, iotat, gridt, ancht, neg1,
         mul_dve, mul_act,
         xp, boxp, negp, pakp, redp, outsbp, psp, ps2p):
    # ------- input DMA: (ch-block x h-quad) tiles spanning all n -------
    # Source addresses span the whole x tensor (the narrow-span per-n
    # tiles made the DGE collapse onto 3 of 16 engines). All 8 tiles are
    # resident (no ring-rotation waits parked at queue heads). A tiny
    # canary rewrite of real tile bytes follows each load on the same
    # queue: per-engine FIFOs mean the canary lands after every packet
    # of the load, closing the cross-instruction completion-count race.
    xt = []
    for c in range(nch):
        t0 = xp.tile([128, N, hqs, W], F32, tag=f"xb0q{c}", name=f"xb0q{c}")
        nc.sync.dma_start(
            out=t0[:, :, :, :],
            in_=x[:, 4:132, c * hqs:(c + 1) * hqs, :].transpose([1, 0, 2, 3]),
        )
        nc.sync.dma_start(
            out=t0[0:16, 0, 0, 0:1], in_=x[0, 4:20, c * hqs:c * hqs + 1, 0:1])
        t1 = xp.tile([128, N, hqs, W], F32, tag=f"xb1q{c}", name=f"xb1q{c}")
        nc.scalar.dma_start(
            out=t1[0:123, :, :, :],
            in_=x[:, 132:255, c * hqs:(c + 1) * hqs, :].transpose([1, 0, 2, 3]),
        )
        nc.scalar.dma_start(
            out=t1[0:16, 0, 0, 0:1],
            in_=x[0, 132:148, c * hqs:c * hqs + 1, 0:1])
        xt.append((t0, t1))

    # per-n c-reduced partial results (all reduces on DVE)
    red = redp.tile([128, N, hl, A], F32, name="red")

    # ---------------- score path ----------------
    pair = 1
    for c in range(nch):
        for n in range(N):
            tps = psp.tile([128, hqs, 256], F32, tag="tps")
            for j in range(hqs):
                nc.tensor.transpose(
                    tps[:, j, 0:128], xt[c][0][:, n, j, :], ident[:, :])
                nc.tensor.transpose(
                    tps[:, j, 128:251], xt[c][1][0:123, n, j, :],
                    ident[:123, :123])

            negsc = negp.tile([128, pair, hqs, A, CLS], F32, tag="negsc")
            ci = 0
            # negscore = (lg * -1) * obj  [128pix, hqs, A, CLS]
            nob = negp.tile([128, hqs, A], F32, tag="nob")
            nc.vector.tensor_scalar_mul(
                nob[:, :, :], tps[:, :, 0:171:85], -1.0)
            if n in mul_act:
                # fused path: ACT reads PSUM, per-(h, a) per-partition scale
                for j in range(hqs):
                    for a in range(A):
                        nc.scalar.mul(
                            negsc[:, ci, j, a, :],
                            tps[:, j, a * 85 + 1:a * 85 + 81],
                            nob[:, j, a:a + 1],
                        )
            elif n in mul_dve:
                # DVE multiplies straight from PSUM (one op per anchor)
                for a in range(A):
                    nc.vector.tensor_tensor(
                        out=negsc[:, ci, :, a, :],
                        in0=tps[:, :, a * 85 + 1:a * 85 + 81],
                        in1=nob[:, :, a:a + 1].broadcast_to([128, hqs, CLS]),
                        op=mybir.AluOpType.mult,
                    )
            else:
                # ACT evicts PSUM -> SBUF in one big copy; Pool multiplies
                # from SBUF (Pool cannot access PSUM)
                tsb = negp.tile([128, hqs, 251], F32, tag="tsb")
                nc.scalar.copy(tsb[:, :, :], tps[:, :, 0:251])
                for a in range(A):
                    nc.gpsimd.tensor_tensor(
                        out=negsc[:, ci, :, a, :],
                        in0=tsb[:, :, a * 85 + 1:a * 85 + 81],
                        in1=nob[:, :, a:a + 1].broadcast_to([128, hqs, CLS]),
                        op=mybir.AluOpType.mult,
                    )

            if c % pair != pair - 1:
                continue
            # pack = (negscore & ~0x3FF) | idx  via (|0x3FF) ^ (idx^0x3FF)
            # (bitwise ops are DVE-only; Pool rejects them)
            packed = pakp.tile([128, pair, hqs, A, CLS], F32, tag="packed")
            iota_ap = iotat[:, n * CLS:(n + 1) * CLS].unsqueeze(1).broadcast_to(
                [128, pair * hqs * A, CLS])
            nc.vector.scalar_tensor_tensor(
                out=packed[:, :, :, :, :].rearrange(
                    "p x h a c -> p (x h a) c").bitcast(U32),
                in0=negsc[:, :, :, :, :].rearrange(
                    "p x h a c -> p (x h a) c").bitcast(U32),
                scalar=bitst[:, 0:1],
                in1=iota_ap,
                op0=mybir.AluOpType.bitwise_or,
                op1=mybir.AluOpType.bitwise_xor,
            )

            # min over classes (axis=X) -> [128, pair*hqs, A]
            c0 = c - pair + 1
            nc.vector.tensor_reduce(
                red[:, n, c0 * hqs:(c + 1) * hqs, :],
                packed[:, :, :, :, :].rearrange("p x h a c -> p (x h) a c"),
                axis=mybir.AxisListType.X,
                op=mybir.AluOpType.min,
            )

        # ---- interleave box path after the first h-quad's chunks ----
        # (load k and its compute emitted together so the 2-buffer ring
        # never overwrites a tile before its emitted consumer)
        if c == 0:
            for k in range(4):
                bt = boxp.tile([bp, bf], F32, tag="bi", name=f"bi{k}")
                nc.scalar.dma_start(
                    out=bt[:, :],
                    in_=x[:, k:255:85, :, :].transpose([1, 0, 2, 3]),
                )
                bo = boxp.tile([bp, bf], F32, tag="bo", name=f"bo{k}")
                if k < 2:
                    nc.scalar.activation(
                        bo[:, :], bt[:, :],
                        mybir.ActivationFunctionType.Sigmoid)
                    nc.gpsimd.tensor_add(bo[:, :], bo[:, :], gridt[k][:, :])
                else:
                    nc.scalar.mul(bo[:, :], bt[:, :], ancht[k - 2][:, :])
                nc.sync.dma_start(
                    out=out[:, k:18:6, :, :].transpose([1, 0, 2, 3]),
                    in_=bo[:, :],
                )

    # ---------------- cross-n min + unpack + output ----------------
    m = redp.tile([128, hl, A], F32, name="m")
    nc.vector.tensor_reduce(
        m[:, :, :],
        red[:, :, :, :].rearrange("p n h a -> p h a n"),
        axis=mybir.AxisListType.X,
        op=mybir.AluOpType.min,
    )

    # vq = packed & ~0x3FF (negated smax, quantized); sarg = low 10 bits
    vq = redp.tile([128, hl * A], F32, name="vq")
    nc.vector.scalar_tensor_tensor(
        out=vq[:, :].bitcast(U32),
        in0=m[:, :, :].rearrange("p h a -> p (h a)").bitcast(U32),
        scalar=bitst[:, 0:1],
        in1=bitst[:, 0:1].broadcast_to([128, hl * A]),
        op0=mybir.AluOpType.bitwise_or, op1=mybir.AluOpType.bitwise_xor,
    )
    sargT = redp.tile([128, hl * A], F32, name="sargT")
    nc.vector.scalar_tensor_tensor(
        out=sargT[:, :].bitcast(U32),
        in0=m[:, :, :].rearrange("p h a -> p (h a)").bitcast(U32),
        scalar=bitst[:, 0:1],
        in1=bitst[:, 1:2].broadcast_to([128, hl * A]),
        op0=mybir.AluOpType.bitwise_and, op1=mybir.AluOpType.bitwise_or,
    )
    nc.vector.scalar_tensor_tensor(
        out=sargT[:, :], in0=sargT[:, :], scalar=1.0,
        in1=bitst[:, 2:3].bitcast(F32).broadcast_to([128, hl * A]),
        op0=mybir.AluOpType.subtract, op1=mybir.AluOpType.mult,
    )

    for a in range(A):
        for t_in, ch_out, scl in ((vq, a * 6 + 4, -1.0),
                                  (sargT, a * 6 + 5, 1.0)):
            t3 = t_in[:, :].rearrange("p (h a) -> p h a", a=A)[:, :, a]
            tpo = ps2p.tile([hl, 128], F32, tag="outps")
            nc.tensor.transpose(tpo[:, :], t3, ident[:, :])
            osb = outsbp.tile([hl, 128], F32, tag="osb")
            if scl == 1.0:
                nc.scalar.copy(osb[:, :], tpo[:, :])
            else:
                nc.scalar.mul(osb[:, :], tpo[:, :], scl)
            si = (ch_out % 6 - 4) * A + a
            nc.sync.dma_start(out=oscr[si, :, :], in_=osb[:, :])
            nc.sync.dma_start(
                out=out[:, ch_out, :, :],
                in_=oscr[si, :, :].unsqueeze(0).broadcast_to([N, hl, W]),
            )


_NC_CACHE = {}


def get_nc(hl=HL):
    if hl not in _NC_CACHE:
        _NC_CACHE[hl] = build_nc(hl)
    return _NC_CACHE[hl]


def make_in_maps(x, hl=HL):
    """Shard the full input along H and build per-core input maps."""
    x = np.ascontiguousarray(x, dtype=np.float32)
    bf = hl * W
    bp = A * N

    gx = np.tile(np.arange(W, dtype=np.float32), hl)             # value = w
    anch_col = np.stack(
        [np.repeat(np.array(ANCHOR_W, np.float32), N),
         np.repeat(np.array(ANCHOR_H, np.float32), N)]
    ).reshape(2, bp, 1)
    iota_bits = np.arange(N * CLS, dtype=np.uint32) ^ 0x3FF
    bits = np.array([0x3FF, 0x3F800000, 0x4B000000, 0],
                    np.uint32)  # masklo, bits(1.0), bits(2^23), unused
    in_maps = []
    ncores = x.shape[2] // hl
    for i in range(ncores):
        grid = np.empty((2, bp, bf), np.float32)
        grid[0] = gx
        gy = np.repeat(np.arange(i * hl, (i + 1) * hl, dtype=np.float32), W)
        grid[1] = gy
        in_maps.append({
            "x": np.ascontiguousarray(x[:, :, i * hl:(i + 1) * hl, :]),
            "grid": grid,
            "anch": anch_col,
            "iota": iota_bits,
            "bits": bits,
        })
    return in_maps


def patch_compile_cache(cache_dir="/tmp/bass_neff_cache"):
    """Cache compiled NEFFs on disk keyed by the BIR hash (compile takes
    minutes; the cache makes repeated runs of an identical graph instant)."""
    import hashlib
    import shutil
    import concourse.bass2jax as b2j

    if getattr(b2j, "_neff_cache_patched", False):
        return
    os.makedirs(cache_dir, exist_ok=True)
    orig = b2j.compile_bir_kernel

    def cached(bir_json, tmpdir, neff_name="file.neff"):
        data = bir_json if isinstance(bir_json, bytes) else str(bir_json).encode()
        key = hashlib.sha256(data).hexdigest()[:32]
        cpath = os.path.join(cache_dir, key + ".neff")
        if os.path.exists(cpath):
            opath = os.path.join(tmpdir, neff_name)
            shutil.copy(cpath, opath)
            return opath
        r = orig(bir_json, tmpdir, neff_name)
        try:
            shutil.copy(r, cpath)
        except OSError:
            pass
        return r

    b2j.compile_bir_kernel = cached
    b2j._neff_cache_patched = True


def kernel(x: np.ndarray) -> np.ndarray:
    from concourse.bass_utils import run_bass_kernel_spmd

    patch_compile_cache()

    nc = get_nc(HL)
    in_maps = make_in_maps(x, HL)
    res = run_bass_kernel_spmd(nc, in_maps, core_ids=list(range(NCORES)))
    return np.concatenate([res.results[i]["out"] for i in range(NCORES)], axis=2)


# revision 34
# speedup vs baseline: 1.0591x; 1.0591x over previous
"""Trainium2 Bass kernel for AnchorProcessor (nms_detection).

Input  x: [8, 255, 128, 128] f32.  Output: [8, 18, 128, 128] f32.
Strategy: shard along H across 8 cores (16 rows each); per-core problem is
fully local (the buggy cross-batch max/argmax reduces over (N, cls), both
on-core), so no collectives.

v2 architecture (v1 in kernel_v1_backup.py ran 168us; DMA floor is ~80us):
  - Full-channel PE transposes: per (n, h) two transposes move channels
    [4:132) and [132:255) of x[n, :, h, :] into one PSUM tile [128pix, 252],
    so obj + all 3 anchors' logits arrive pixel-major together (256 big
    matmuls total vs 406 small ones in v1, and no separate obj transposes).
  - Input DMA as 16 big n-major tiles [128ch, 16h, 128w] with 8KB
    descriptors; compute pipelines per (n, h-quad) chunk behind the DMA.
  - negscore = (lg * -1) * obj is ONE scalar_tensor_tensor per chunk (obj
    columns broadcast along c) instead of v1's ACT/DVE per-(n,a) splits.
  - Work spread across engines (DVE was 116us in v1, gpsimd idle):
      mul: DVE n0-4, Pool n5, ACT n6-7 (per-partition obj scale)
      pack (bit-pack value|index): Pool (gpsimd)
      c-reduce (min over 80 classes, axis=X): DVE only (Pool can't do X)
      cross-n min + unpack: DVE (tiny)
      box path: ACT sigmoid/scale + Pool grid adds in a [96, 512] layout.
  - Same bit-pack argmax trick as v1: packed = (negscore&~0x3FF) | idx via
    (or,xor) stt; one min-reduce gives smax (2^-13 rel quant) + argmax.
"""

import os
import sys

for _p in ("/opt/trn_rl_repo", "/root/.axon_site/_ro/trn_rl_repo"):
    if _p not in sys.path:
        sys.path.append(_p)

import numpy as np

from concourse import bacc, masks, mybir
from concourse.tile import TileContext

N = 8          # batch
A = 3          # anchors
CLS = 80       # classes per anchor
W = 128        # width
HL = 16        # local H rows per core (128 / 8 cores)
NCORES = 8

ANCHOR_W = (116.0, 156.0, 373.0)
ANCHOR_H = (90.0, 198.0, 326.0)

F32 = mybir.dt.float32
U32 = mybir.dt.uint32

# engine assignment knobs (tunable via env for experiments)
def _env_tuple(name, default):
    v = os.environ.get(name)
    if v is None:
        return default
    return tuple(int(s) for s in v.split(",") if s != "")


MUL_DVE_NS = _env_tuple("MUL_DVE", ())   # mul on DVE (from SBUF copy)
MUL_ACT_NS = _env_tuple("MUL_ACT", (6, 7))  # mul fused on ACT (from PSUM)


def build_nc(hl=HL, reps=1, mul_dve=MUL_DVE_NS, mul_act=MUL_ACT_NS):
    import contextlib

    hqs = 4 if hl % 4 == 0 else hl     # h rows per chunk
    nch = hl // hqs                    # chunks per n
    bf = hl * W                        # box free size
    bp = A * N                         # box partitions

    nc = bacc.Bacc("TRN2", target_bir_lowering=False, debug=False)

    x = nc.declare_dram_parameter("x", [N, 255, hl, W], F32, isOutput=False)
    grid = nc.declare_dram_parameter("grid", [2, bp, bf], F32, isOutput=False)
    anch = nc.declare_dram_parameter("anch", [2, bp, 1], F32, isOutput=False)
    iota = nc.declare_dram_parameter("iota", [N * CLS], U32, isOutput=False)
    bits = nc.declare_dram_parameter("bits", [4], U32, isOutput=False)
    out = nc.declare_dram_parameter("out", [N, A * 6, hl, W], F32, isOutput=True)
    oscr = nc.dram_tensor("oscratch", [A * 2, hl, W], F32)

    with TileContext(nc) as tc:
        with (
            tc.tile_pool(name="const", bufs=1) as constp,
            tc.tile_pool(name="xt", bufs=1) as xp,
            tc.tile_pool(name="box", bufs=2) as boxp,
            tc.tile_pool(name="neg", bufs=2) as negp,
            tc.tile_pool(name="pak", bufs=2) as pakp,
            tc.tile_pool(name="red", bufs=1) as redp,
            tc.tile_pool(name="outsb", bufs=6) as outsbp,
            tc.tile_pool(name="ps", bufs=3, space="PSUM") as psp,
            tc.tile_pool(name="ps2", bufs=2, space="PSUM") as ps2p,
        ):
            ident = constp.tile([128, 128], F32)
            masks.make_identity(nc, ident[:, :])
            neg1 = constp.tile([128, 1], F32)
            nc.gpsimd.memset(neg1[:, :], -1.0)

            bitst = constp.tile([128, 4], U32)
            nc.sync.dma_start(
                out=bitst[:, :],
                in_=bits[:].unsqueeze(0).broadcast_to([128, 4]),
            )
            iotat = constp.tile([128, N * CLS], U32)
            nc.scalar.dma_start(
                out=iotat[:, :],
                in_=iota[:].unsqueeze(0).broadcast_to([128, N * CLS]),
            )
            gridt = [constp.tile([bp, bf], F32, name=f"grid{g}") for g in range(2)]
            ancht = [constp.tile([bp, 1], F32, name=f"anch{g}") for g in range(2)]
            for g in range(2):
                nc.scalar.dma_start(out=gridt[g][:, :], in_=grid[g, :, :])
                nc.scalar.dma_start(out=ancht[g][:, :], in_=anch[g, :, :])
            # (scalar-queue DMAs above are small; all bulk loads go on the
            # sync queue, whose descriptors spread across all 16 DMA engines)

            loop_cm = (
                tc.For_i(0, reps, 1, hint_engines=(mybir.EngineType.PE,))
                if reps > 1 else contextlib.nullcontext()
            )
            with loop_cm:
                body(nc, tc, x, out, oscr, hl, hqs, nch, bf, bp,
                     ident, bitst, iotat, gridt, ancht, neg1,
                     mul_dve, mul_act,
                     xp, boxp, negp, pakp, redp, outsbp, psp, ps2p)

    nc.compile()
    return nc


def body(nc, tc, x, out, oscr, hl, hqs, nch, bf, bp,
         ident, bitst, iotat, gridt, ancht, neg1,
         mul_dve, mul_act,
         xp, boxp, negp, pakp, redp, outsbp, psp, ps2p):
    # ------- input DMA: (ch-block x h-quad) tiles spanning all n -------
    # Source addresses span the whole x tensor (the narrow-span per-n
    # tiles made the DGE collapse onto 3 of 16 engines). All 8 tiles are
    # resident (no ring-rotation waits parked at queue heads). A tiny
    # canary rewrite of real tile bytes follows each load on the same
    # queue: per-engine FIFOs mean the canary lands after every packet
    # of the load, closing the cross-instruction completion-count race.
    xt = []
    for c in range(nch):
        t0 = xp.tile([128, N, hqs, W], F32, tag=f"xb0q{c}", name=f"xb0q{c}")
        nc.sync.dma_start(
            out=t0[:, :, :, :],
            in_=x[:, 4:132, c * hqs:(c + 1) * hqs, :].transpose([1, 0, 2, 3]),
        )
        nc.sync.dma_start(
            out=t0[0:16, 0, 0, 0:1], in_=x[0, 4:20, c * hqs:c * hqs + 1, 0:1])
        t1 = xp.tile([128, N, hqs, W], F32, tag=f"xb1q{c}", name=f"xb1q{c}")
        nc.sync.dma_start(
            out=t1[0:123, :, :, :],
            in_=x[:, 132:255, c * hqs:(c + 1) * hqs, :].transpose([1, 0, 2, 3]),
        )
        nc.sync.dma_start(
            out=t1[0:16, 0, 0, 0:1],
            in_=x[0, 132:148, c * hqs:c * hqs + 1, 0:1])
        xt.append((t0, t1))

    # per-n c-reduced partial results (all reduces on DVE)
    red = redp.tile([128, N, hl, A], F32, name="red")

    # ---------------- score path ----------------
    pair = 1
    for c in range(nch):
        for n in range(N):
            tps = psp.tile([128, hqs, 256], F32, tag="tps")
            for j in range(hqs):
                nc.tensor.transpose(
                    tps[:, j, 0:128], xt[c][0][:, n, j, :], ident[:, :])
                nc.tensor.transpose(
                    tps[:, j, 128:251], xt[c][1][0:123, n, j, :],
                    ident[:123, :123])

            negsc = negp.tile([128, pair, hqs, A, CLS], F32, tag="negsc")
            ci = 0
            # negscore = (lg * -1) * obj  [128pix, hqs, A, CLS]
            nob = negp.tile([128, hqs, A], F32, tag="nob")
            nc.vector.tensor_scalar_mul(
                nob[:, :, :], tps[:, :, 0:171:85], -1.0)
            if n in mul_act:
                # fused path: ACT reads PSUM, per-(h, a) per-partition scale
                for j in range(hqs):
                    for a in range(A):
                        nc.scalar.mul(
                            negsc[:, ci, j, a, :],
                            tps[:, j, a * 85 + 1:a * 85 + 81],
                            nob[:, j, a:a + 1],
                        )
            elif n in mul_dve:
                # DVE multiplies straight from PSUM (one op per anchor)
                for a in range(A):
                    nc.vector.tensor_tensor(
                        out=negsc[:, ci, :, a, :],
                        in0=tps[:, :, a * 85 + 1:a * 85 + 81],
                        in1=nob[:, :, a:a + 1].broadcast_to([128, hqs, CLS]),
                        op=mybir.AluOpType.mult,
                    )
            else:
                # ACT evicts PSUM -> SBUF in one big copy; Pool multiplies
                # from SBUF (Pool cannot access PSUM)
                tsb = negp.tile([128, hqs, 251], F32, tag="tsb")
                nc.scalar.copy(tsb[:, :, :], tps[:, :, 0:251])
                for a in range(A):
                    nc.gpsimd.tensor_tensor(
                        out=negsc[:, ci, :, a, :],
                        in0=tsb[:, :, a * 85 + 1:a * 85 + 81],
                        in1=nob[:, :, a:a + 1].broadcast_to([128, hqs, CLS]),
                        op=mybir.AluOpType.mult,
                    )

            if c % pair != pair - 1:
                continue
            # pack = (negscore & ~0x3FF) | idx  via (|0x3FF) ^ (idx^0x3FF)
            # (bitwise ops are DVE-only; Pool rejects them)
            packed = pakp.tile([128, pair, hqs, A, CLS], F32, tag="packed")
            iota_ap = iotat[:, n * CLS:(n + 1) * CLS].unsqueeze(1).broadcast_to(
                [128, pair * hqs * A, CLS])
            nc.vector.scalar_tensor_tensor(
                out=packed[:, :, :, :, :].rearrange(
                    "p x h a c -> p (x h a) c").bitcast(U32),
                in0=negsc[:, :, :, :, :].rearrange(
                    "p x h a c -> p (x h a) c").bitcast(U32),
                scalar=bitst[:, 0:1],
                in1=iota_ap,
                op0=mybir.AluOpType.bitwise_or,
                op1=mybir.AluOpType.bitwise_xor,
            )

            # min over classes (axis=X) -> [128, pair*hqs, A]
            c0 = c - pair + 1
            nc.vector.tensor_reduce(
                red[:, n, c0 * hqs:(c + 1) * hqs, :],
                packed[:, :, :, :, :].rearrange("p x h a c -> p (x h) a c"),
                axis=mybir.AxisListType.X,
                op=mybir.AluOpType.min,
            )

        # ---- interleave box path after the first h-quad's chunks ----
        # (load k and its compute emitted together so the 2-buffer ring
        # never overwrites a tile before its emitted consumer)
        if c == 0:
            for k in range(4):
                bt = boxp.tile([bp, bf], F32, tag="bi", name=f"bi{k}")
                nc.scalar.dma_start(
                    out=bt[:, :],
                    in_=x[:, k:255:85, :, :].transpose([1, 0, 2, 3]),
                )
                bo = boxp.tile([bp, bf], F32, tag="bo", name=f"bo{k}")
                if k < 2:
                    nc.scalar.activation(
                        bo[:, :], bt[:, :],
                        mybir.ActivationFunctionType.Sigmoid)
                    nc.gpsimd.tensor_add(bo[:, :], bo[:, :], gridt[k][:, :])
                else:
                    nc.scalar.mul(bo[:, :], bt[:, :], ancht[k - 2][:, :])
                nc.scalar.dma_start(
                    out=out[:, k:18:6, :, :].transpose([1, 0, 2, 3]),
                    in_=bo[:, :],
                )

    # ---------------- cross-n min + unpack + output ----------------
    m = redp.tile([128, hl, A], F32, name="m")
    nc.vector.tensor_reduce(
        m[:, :, :],
        red[:, :, :, :].rearrange("p n h a -> p h a n"),
        axis=mybir.AxisListType.X,
        op=mybir.AluOpType.min,
    )

    # vq = packed & ~0x3FF (negated smax, quantized); sarg = low 10 bits
    vq = redp.tile([128, hl * A], F32, name="vq")
    nc.vector.scalar_tensor_tensor(
        out=vq[:, :].bitcast(U32),
        in0=m[:, :, :].rearrange("p h a -> p (h a)").bitcast(U32),
        scalar=bitst[:, 0:1],
        in1=bitst[:, 0:1].broadcast_to([128, hl * A]),
        op0=mybir.AluOpType.bitwise_or, op1=mybir.AluOpType.bitwise_xor,
    )
    sargT = redp.tile([128, hl * A], F32, name="sargT")
    nc.vector.scalar_tensor_tensor(
        out=sargT[:, :].bitcast(U32),
        in0=m[:, :, :].rearrange("p h a -> p (h a)").bitcast(U32),
        scalar=bitst[:, 0:1],
        in1=bitst[:, 1:2].broadcast_to([128, hl * A]),
        op0=mybir.AluOpType.bitwise_and, op1=mybir.AluOpType.bitwise_or,
    )
    nc.vector.scalar_tensor_tensor(
        out=sargT[:, :], in0=sargT[:, :], scalar=1.0,
        in1=bitst[:, 2:3].bitcast(F32).broadcast_to([128, hl * A]),
        op0=mybir.AluOpType.subtract, op1=mybir.AluOpType.mult,
    )

    for a in range(A):
        for t_in, ch_out, scl in ((vq, a * 6 + 4, -1.0),
                                  (sargT, a * 6 + 5, 1.0)):
            t3 = t_in[:, :].rearrange("p (h a) -> p h a", a=A)[:, :, a]
            tpo = ps2p.tile([hl, 128], F32, tag="outps")
            nc.tensor.transpose(tpo[:, :], t3, ident[:, :])
            osb = outsbp.tile([hl, 128], F32, tag="osb")
            if scl == 1.0:
                nc.scalar.copy(osb[:, :], tpo[:, :])
            else:
                nc.scalar.mul(osb[:, :], tpo[:, :], scl)
            si = (ch_out % 6 - 4) * A + a
            nc.scalar.dma_start(out=oscr[si, :, :], in_=osb[:, :])
            nc.scalar.dma_start(
                out=out[:, ch_out, :, :],
                in_=oscr[si, :, :].unsqueeze(0).broadcast_to([N, hl, W]),
            )


_NC_CACHE = {}


def get_nc(hl=HL):
    if hl not in _NC_CACHE:
        _NC_CACHE[hl] = build_nc(hl)
    return _NC_CACHE[hl]


def make_in_maps(x, hl=HL):
    """Shard the full input along H and build per-core input maps."""
    x = np.ascontiguousarray(x, dtype=np.float32)
    bf = hl * W
    bp = A * N

    gx = np.tile(np.arange(W, dtype=np.float32), hl)             # value = w
    anch_col = np.stack(
        [np.repeat(np.array(ANCHOR_W, np.float32), N),
         np.repeat(np.array(ANCHOR_H, np.float32), N)]
    ).reshape(2, bp, 1)
    iota_bits = np.arange(N * CLS, dtype=np.uint32) ^ 0x3FF
    bits = np.array([0x3FF, 0x3F800000, 0x4B000000, 0],
                    np.uint32)  # masklo, bits(1.0), bits(2^23), unused
    in_maps = []
    ncores = x.shape[2] // hl
    for i in range(ncores):
        grid = np.empty((2, bp, bf), np.float32)
        grid[0] = gx
        gy = np.repeat(np.arange(i * hl, (i + 1) * hl, dtype=np.float32), W)
        grid[1] = gy
        in_maps.append({
            "x": np.ascontiguousarray(x[:, :, i * hl:(i + 1) * hl, :]),
            "grid": grid,
            "anch": anch_col,
            "iota": iota_bits,
            "bits": bits,
        })
    return in_maps


def patch_compile_cache(cache_dir="/tmp/bass_neff_cache"):
    """Cache compiled NEFFs on disk keyed by the BIR hash (compile takes
    minutes; the cache makes repeated runs of an identical graph instant)."""
    import hashlib
    import shutil
    import concourse.bass2jax as b2j

    if getattr(b2j, "_neff_cache_patched", False):
        return
    os.makedirs(cache_dir, exist_ok=True)
    orig = b2j.compile_bir_kernel

    def cached(bir_json, tmpdir, neff_name="file.neff"):
        data = bir_json if isinstance(bir_json, bytes) else str(bir_json).encode()
        key = hashlib.sha256(data).hexdigest()[:32]
        cpath = os.path.join(cache_dir, key + ".neff")
        if os.path.exists(cpath):
            opath = os.path.join(tmpdir, neff_name)
            shutil.copy(cpath, opath)
            return opath
        r = orig(bir_json, tmpdir, neff_name)
        try:
            shutil.copy(r, cpath)
        except OSError:
            pass
        return r

    b2j.compile_bir_kernel = cached
    b2j._neff_cache_patched = True


def kernel(x: np.ndarray) -> np.ndarray:
    from concourse.bass_utils import run_bass_kernel_spmd

    patch_compile_cache()

    nc = get_nc(HL)
    in_maps = make_in_maps(x, HL)
    res = run_bass_kernel_spmd(nc, in_maps, core_ids=list(range(NCORES)))
    return np.concatenate([res.results[i]["out"] for i in range(NCORES)], axis=2)


# revision 37
# speedup vs baseline: 1.9177x; 1.8106x over previous
"""Trainium2 Bass kernel for AnchorProcessor (nms_detection).

Input  x: [8, 255, 128, 128] f32.  Output: [8, 18, 128, 128] f32.
Strategy: shard along H across 8 cores (16 rows each). Each core's problem is
fully local (the buggy cross-batch max/argmax reduces over (N, cls) which are
both on-core), so there are no collectives.

Per core (N=8, A=3, cls=80, HL=16, W=128), score path works on NEGATED
scores (min-reduce) with a bit-packed value|index key:
  - logits preloaded per (anchor, 4-row block) as [80, N, 4, W] tiles with
    2KB DMA descriptors, alternating sync/scalar DMA queues.
  - PE transposes each (n, row) into PSUM [128pix x 80c].
  - negscore = lgps * (-obj): the per-partition-scalar multiply is split
    2/8 on ACT (per-n scalar.mul) and 6/8 on DVE (tensor_mul, broadcast).
  - pack (one J-batched DVE scalar_tensor_tensor):
      packed = (negscore & 0xFFFFFC00) | iota_bits   (iota = flat n*80+c)
    monotone in negscore (10-bit index in the cleared mantissa low bits),
    so ONE min-reduce yields both min(negscore) = -smax (quantized to
    2^-13 rel) and the argmax index in the low 10 bits.
  - tiny per-anchor extraction ops unpack smax / sarg; results are
    PE-transposed back and broadcast to all 8 batch entries.
(tensor_tensor_reduce with op1=max hangs on this silicon; op1=min fails
too - probed both. The eq+iota exact argmax costs a full extra DVE pass;
the packed quantization error (2^-13 rel on smax, ~1e-4 of pixels get a
tie-broken argmax) is far inside the 2e-2 gate.)
"""

import os
import sys

for _p in ("/opt/trn_rl_repo", "/root/.axon_site/_ro/trn_rl_repo"):
    if _p not in sys.path:
        sys.path.append(_p)

import numpy as np

from concourse import bacc, masks, mybir
from concourse.tile import TileContext

N = 8          # batch
A = 3          # anchors
CLS = 80       # classes per anchor
W = 128        # width
HL = 16        # local H rows per core (128 / 8 cores)
NCORES = 8

ANCHOR_W = (116.0, 156.0, 373.0)
ANCHOR_H = (90.0, 198.0, 326.0)

F32 = mybir.dt.float32
FR = mybir.dt.float32r
U32 = mybir.dt.uint32

N_ACT = 2      # batch entries whose obj-multiply runs on ACT (rest on DVE)


def build_nc(hl=HL, reps=1, use_ttr=False, n_act=N_ACT, fr=False):
    """Build the single-core graph (same SPMD graph on all 8 cores)."""
    import contextlib
    pix = hl * W           # pixels per core
    ch = hl                # one chunk per local h-row (128 pixels each)

    nc = bacc.Bacc("TRN2", target_bir_lowering=False, debug=False)

    x = nc.declare_dram_parameter("x", [N, 255, hl, W], F32, isOutput=False)
    grid = nc.declare_dram_parameter("grid", [2, A * N, pix], F32, isOutput=False)
    anch = nc.declare_dram_parameter("anch", [2, A * N, 1], F32, isOutput=False)
    iota = nc.declare_dram_parameter("iota", [N * CLS], U32, isOutput=False)
    bits = nc.declare_dram_parameter("bits", [4], U32, isOutput=False)
    out = nc.declare_dram_parameter("out", [N, A * 6, hl, W], F32, isOutput=True)
    oscr = nc.dram_tensor("oscratch", [A * 2, hl, W], F32)

    with TileContext(nc) as tc:
        with (
            tc.tile_pool(name="const", bufs=1) as constp,
            tc.tile_pool(name="box", bufs=2) as boxp,
            tc.tile_pool(name="objsb", bufs=1) as objsbp,
            tc.tile_pool(name="lg", bufs=3) as lgp,
            tc.tile_pool(name="score", bufs=2) as scorep,
            tc.tile_pool(name="res", bufs=2) as resp,
            tc.tile_pool(name="outsb", bufs=3) as outsbp,
            tc.tile_pool(name="ps", bufs=3, space="PSUM") as psp,
            tc.tile_pool(name="ps2", bufs=1, space="PSUM") as ps2p,
        ):
            ident = constp.tile([128, 128], F32)
            masks.make_identity(nc, ident[:, :])

            gridt = [constp.tile([A * N, pix], F32, name=f"grid{g}", tag=f"grid{g}") for g in range(2)]
            ancht = [constp.tile([A * N, 1], F32, name=f"anch{g}", tag=f"anch{g}") for g in range(2)]
            for g in range(2):
                nc.scalar.dma_start(out=gridt[g][:, :], in_=grid[g, :, :])
                nc.scalar.dma_start(out=ancht[g][:, :], in_=anch[g, :, :])

            iotat = constp.tile([128, N * CLS], U32)
            nc.scalar.dma_start(
                out=iotat[:, :],
                in_=iota[:].unsqueeze(0).broadcast_to([128, N * CLS]),
            )
            bitst = constp.tile([128, 4], U32)
            nc.scalar.dma_start(
                out=bitst[:, :],
                in_=bits[:].unsqueeze(0).broadcast_to([128, 4]),
            )

            loop_cm = (
                tc.For_i(0, reps, 1, hint_engines=(mybir.EngineType.PE,))
                if reps > 1 else contextlib.nullcontext()
            )
            with loop_cm:
                body(nc, tc, x, out, oscr, pix, ch, hl,
                     ident, gridt, ancht, iotat, bitst, n_act, fr,
                     constp, boxp, objsbp, lgp, scorep, resp, outsbp, psp, ps2p)

    nc.compile()
    return nc


def body(nc, tc, x, out, oscr, pix, ch, hl, ident, gridt, ancht, iotat, bitst,
         n_act, fr,
         constp, boxp, objsbp, lgp, scorep, resp, outsbp, psp, ps2p):
    # objectness planes, rows ordered (a, n) a-major (scalar queue: keep the
    # sync queue free so the first logit block lands ASAP)
    objt = boxp.tile([A * N, pix], F32, tag="objt", name="objt")
    nc.scalar.dma_start(
        out=objt[:, :],
        in_=x[:, 4:255:85, :, :].transpose([1, 0, 2, 3]),
    )

    # NEGATED transposed objectness: objTn[pix, chunk, a, n] = -obj
    objTn = objsbp.tile([128, ch, A, N], F32)
    for j in range(ch):
        ops = ps2p.tile([128, A * N], F32)
        nc.tensor.transpose(
            ops[:, :], objt[:, j * 128:(j + 1) * 128], ident[:A * N, :A * N]
        )
        nc.scalar.mul(objTn[:, j, :, :], ops[:, :], -1.0)

    # ---------------- score path (negated, bit-packed argmax) --------------
    HB = 4 if ch % 4 == 0 else ch      # rows per logit preload block
    JB = 4 if ch % 4 == 0 else 1       # rows per pack/reduce batch
    for a in range(A):
        negsmaxT = resp.tile([128, ch], F32, tag="smaxT")
        for hb in range(0, ch, HB):
            # preload logits for all n, HB rows: 2KB contiguous descriptors
            lg = lgp.tile([80, N, HB, W], F32)
            eng = nc.sync if (hb // HB) % 2 == 0 else nc.scalar
            eng.dma_start(
                out=lg[:, :, :, :],
                in_=x[:, a * 85 + 5:a * 85 + 85, hb:hb + HB, :].transpose(
                    [1, 0, 2, 3]),
            )
            for rg in range(0, HB, JB):
                scoreg = scorep.tile([128, JB, N, CLS], F32, tag="negscore")
                packed = scorep.tile([128, JB, N, CLS], F32, tag="packed")
                for jj in range(JB):
                    r = rg + jj
                    j = hb + r
                    # transpose each n into PSUM: lgps[pix, n, c]
                    # (fp32r transpose mode: 1.5 PE cycles/row vs 2.0 fp32)
                    lgps = psp.tile([128, N, 128], F32)
                    for n in (0, 4, 1, 5, 2, 6, 3, 7):
                        if fr:
                            nc.tensor.transpose(
                                lgps[:, n, 0:80].bitcast(FR),
                                lg[:, n, r, :].bitcast(FR),
                                ident[:80, :80].bitcast(FR),
                            )
                        else:
                            nc.tensor.transpose(
                                lgps[:, n, 0:80], lg[:, n, r, :],
                                ident[:80, :80]
                            )
                    # negscore = lgps * (-obj): ACT for n < n_act (per-n
                    # per-partition scalar), DVE for the rest (broadcast).
                    for n in range(n_act):
                        nc.scalar.mul(
                            scoreg[:, jj, n, :], lgps[:, n, 0:80],
                            objTn[:, j, a, n:n + 1],
                        )
                    if n_act < N:
                        nobj_b = objTn[:, j, a, n_act:].unsqueeze(2).broadcast_to(
                            [128, N - n_act, CLS])
                        nc.vector.tensor_mul(
                            scoreg[:, jj, n_act:, :],
                            lgps[:, n_act:, 0:80], nobj_b)
                # pack = (negscore | 0x3FF) ^ (0x3FF ^ iota_bits)  (J-batched)
                # == (negscore & ~0x3FF) | iota, without NaN constants
                nc.vector.scalar_tensor_tensor(
                    out=packed[:, :, :, :].bitcast(U32),
                    in0=scoreg[:, :, :, :].bitcast(U32),
                    scalar=bitst[:, 0:1],
                    in1=iotat[:, :].rearrange(
                        "p (n c) -> p n c", n=N).unsqueeze(1).broadcast_to(
                        [128, JB, N, CLS]),
                    op0=mybir.AluOpType.bitwise_or,
                    op1=mybir.AluOpType.bitwise_xor,
                )
                nc.vector.tensor_reduce(
                    negsmaxT[:, hb + rg:hb + rg + JB],
                    packed[:, :, :, :].rearrange("p j n c -> p j (n c)"),
                    axis=mybir.AxisListType.X,
                    op=mybir.AluOpType.min,
                )

        # unpack: vq = (packed | 0x3FF) ^ 0x3FF  (= packed & ~0x3FF);
        # sarg = (((packed & 0x3FF) | bits(1.0)) - 1.0) * 2^23
        vq = resp.tile([128, ch], F32, tag="vq")
        nc.vector.scalar_tensor_tensor(
            out=vq[:, :].bitcast(U32), in0=negsmaxT[:, :].bitcast(U32),
            scalar=bitst[:, 0:1],
            in1=bitst[:, 0:1].broadcast_to([128, ch]),
            op0=mybir.AluOpType.bitwise_or, op1=mybir.AluOpType.bitwise_xor,
        )
        sargT = resp.tile([128, ch], F32, tag="sargT")
        nc.vector.scalar_tensor_tensor(
            out=sargT[:, :].bitcast(U32), in0=negsmaxT[:, :].bitcast(U32),
            scalar=bitst[:, 0:1],
            in1=bitst[:, 1:2].broadcast_to([128, ch]),
            op0=mybir.AluOpType.bitwise_and, op1=mybir.AluOpType.bitwise_or,
        )
        nc.vector.scalar_tensor_tensor(
            out=sargT[:, :], in0=sargT[:, :], scalar=1.0,
            in1=bitst[:, 2:3].bitcast(F32).broadcast_to([128, ch]),
            op0=mybir.AluOpType.subtract, op1=mybir.AluOpType.mult,
        )

        for t_in, ch_out, scl in ((vq, a * 6 + 4, -1.0),
                                  (sargT, a * 6 + 5, 1.0)):
            tps = ps2p.tile([hl, 128], F32, tag="outps")
            nc.tensor.transpose(tps[:, :], t_in[:, :], ident[:, :])
            osb = outsbp.tile([hl, 128], F32, tag="osb")
            if scl == 1.0:
                nc.scalar.copy(osb[:, :], tps[:, :])
            else:
                nc.scalar.mul(osb[:, :], tps[:, :], scl)
            si = (ch_out % 6 - 4) * A + a
            nc.sync.dma_start(out=oscr[si, :, :], in_=osb[:, :])
            nc.sync.dma_start(
                out=out[:, ch_out, :, :],
                in_=oscr[si, :, :].unsqueeze(0).broadcast_to(
                    [N, hl, W]),
            )

    # ---------------- box path (natural layout) ----------------
    for k in (0, 1, 2, 3):
        t = boxp.tile([A * N, pix], F32, tag="boxt", name=f"bx{k}")
        nc.scalar.dma_start(
            out=t[:, :],
            in_=x[:, k:255:85, :, :].transpose([1, 0, 2, 3]),
        )
        o = boxp.tile([A * N, pix], F32, tag="boxo", name=f"bo{k}")
        if k < 2:
            nc.scalar.activation(
                o[:, :], t[:, :], mybir.ActivationFunctionType.Sigmoid
            )
            # + gx (rows 0..23) or + gy (rows 24..47) on the idle gpsimd
            nc.vector.tensor_add(o[:, :], o[:, :], gridt[k][:, :])
        else:
            # per-partition anchor const via ACT scale
            nc.scalar.mul(o[:, :], t[:, :], ancht[k - 2][:, :])
        nc.sync.dma_start(
            out=out[:, k:18:6, :, :].transpose([1, 0, 2, 3]),
            in_=o[:, :],
        )


_NC_CACHE = {}


def get_nc(hl=HL, n_act=None, fr=None):
    if n_act is None:
        n_act = int(os.environ.get("NACT", str(N_ACT)))
    if fr is None:
        fr = os.environ.get("FR", "0") == "1"
    key = (hl, n_act, fr)
    if key not in _NC_CACHE:
        _NC_CACHE[key] = build_nc(hl, n_act=n_act, fr=fr)
    return _NC_CACHE[key]


def make_in_maps(x, hl=HL):
    """Shard the full input along H and build per-core input maps."""
    x = np.ascontiguousarray(x, dtype=np.float32)
    pix = hl * W
    gx = np.tile(np.arange(W, dtype=np.float32), hl)          # value = w
    anch_col = np.stack(
        [np.repeat(np.array(ANCHOR_W, np.float32), N),
         np.repeat(np.array(ANCHOR_H, np.float32), N)]
    ).reshape(2, A * N, 1)
    iota_bits = np.arange(N * CLS, dtype=np.uint32) ^ 0x3FF
    bits = np.array([0x3FF, 0x3F800000, 0x4B000000, 0],
                    np.uint32)  # masklo, bits(1.0), bits(2^23), unused
    in_maps = []
    ncores = x.shape[2] // hl
    for i in range(ncores):
        gy = np.repeat(np.arange(i * hl, (i + 1) * hl, dtype=np.float32), W)
        grid = np.empty((2, A * N, pix), np.float32)
        grid[0] = gx
        grid[1] = gy
        in_maps.append({
            "x": np.ascontiguousarray(x[:, :, i * hl:(i + 1) * hl, :]),
            "grid": grid,
            "anch": anch_col,
            "iota": iota_bits,
            "bits": bits,
        })
    return in_maps


def patch_compile_cache(cache_dir="/tmp/bass_neff_cache"):
    """Cache compiled NEFFs on disk keyed by the BIR hash (compile takes
    minutes; the cache makes repeated runs of an identical graph instant)."""
    import hashlib
    import shutil
    import concourse.bass2jax as b2j

    if getattr(b2j, "_neff_cache_patched", False):
        return
    os.makedirs(cache_dir, exist_ok=True)
    orig = b2j.compile_bir_kernel

    def cached(bir_json, tmpdir, neff_name="file.neff"):
        data = bir_json if isinstance(bir_json, bytes) else str(bir_json).encode()
        key = hashlib.sha256(data).hexdigest()[:32]
        cpath = os.path.join(cache_dir, key + ".neff")
        if os.path.exists(cpath):
            opath = os.path.join(tmpdir, neff_name)
            shutil.copy(cpath, opath)
            return opath
        r = orig(bir_json, tmpdir, neff_name)
        try:
            shutil.copy(r, cpath)
        except OSError:
            pass
        return r

    b2j.compile_bir_kernel = cached
    b2j._neff_cache_patched = True


def kernel(x: np.ndarray) -> np.ndarray:
    from concourse.bass_utils import run_bass_kernel_spmd

    patch_compile_cache()

    nc = get_nc(HL)
    in_maps = make_in_maps(x, HL)
    res = run_bass_kernel_spmd(nc, in_maps, core_ids=list(range(NCORES)))
    return np.concatenate([res.results[i]["out"] for i in range(NCORES)], axis=2)

